# revision 5
# baseline (speedup 1.0000x reference)
"""Trainium2 Bass kernel for a 16-expert top-2 MoE layer with shared expert.

Sharding (8 cores):
  - Routed experts: expert-parallel, 2 experts per core, host-side token
    dispatch (gather) with a static per-expert capacity of C tokens.
  - Shared expert: tensor-parallel over the 4096-wide intermediate dim
    (512 per core); partial outputs summed on host.
  - Router runs on host (tiny: [2048,1024]@[1024,16]).

Matmuls run in fp32r (TF32) at full PE rate; all weights/activations are
pre-rounded to TF32 on host, intermediates are rounded by the engines when
written to fp32r tiles.

Program order interleaves shared-expert chunks between the two routed
experts so the PE stays busy while the next expert's weights stream in.
Streamed routed weights go on the sync HWDGE queue; everything else
(activations, resident shared weights, outputs) on the scalar HWDGE queue.
"""
import sys
sys.path.insert(0, "/opt/trn_rl_repo")
import numpy as np
from concourse import bacc, mybir
from concourse import tile
from concourse import bass_utils

# Problem shape (hardcoded per contract)
B, T, D = 2, 1024, 1024
N = B * T                # 2048 tokens
E = 16                   # routed experts
H = 2048                 # expert intermediate
K = 2                    # top-k
HS = 4096                # shared intermediate
NCORES = 8
EPC = E // NCORES        # experts per core = 2
HSS = HS // NCORES       # shared intermediate slice per core = 512

P = 128
C = 288                  # per-expert token capacity (mean load is 256)
CT = (C + P - 1) // P    # 3 token tiles per expert (last one 32 rows)
DT = D // P              # 8 contraction slices over D
HT = H // P              # 16 H tiles
HST = HSS // P           # 4 shared-H tiles per core
SCH = 512                # shared-expert token chunk (moving dim)
NCH = N // SCH           # 4 chunks
DN = D // 512            # 2 output free tiles of 512

F32 = mybir.dt.float32
F32R = mybir.dt.float32r
AF = mybir.ActivationFunctionType

_CACHED = {}


def _round_tf32(a: np.ndarray) -> np.ndarray:
    """Round-to-nearest-even fp32 -> tf32 (10 mantissa bits), as fp32 bits."""
    u = np.ascontiguousarray(a, dtype=np.float32).view(np.uint32).astype(np.uint64)
    u = u + ((u >> 13) & 1) + 0x0FFF
    return (u & 0xFFFFE000).astype(np.uint32).view(np.float32)


def _mtiles():
    """Token tiles of (offset, width) covering C in 128-row steps."""
    out = []
    off = 0
    while off < C:
        out.append((off, min(P, C - off)))
        off += P
    return out


def _build_nc():
    nc = bacc.Bacc("TRN2", target_bir_lowering=False, debug=False,
                   num_devices=NCORES)

    # ---- DRAM I/O (per-core) ----
    xg = nc.dram_tensor("xg", [EPC, P, DT * C], F32R, kind="ExternalInput").ap()
    gt = nc.dram_tensor("gt", [EPC, CT, P, 1], F32, kind="ExternalInput").ap()
    w1 = nc.dram_tensor("w1", [EPC, HT, P, DT * P], F32R, kind="ExternalInput").ap()
    w11 = nc.dram_tensor("w11", [EPC, HT, P, DT * P], F32R, kind="ExternalInput").ap()
    w2 = nc.dram_tensor("w2", [EPC, HT, P, D], F32R, kind="ExternalInput").ap()
    b1 = nc.dram_tensor("b1", [EPC, HT, P, 1], F32, kind="ExternalInput").ap()
    b11 = nc.dram_tensor("b11", [EPC, HT, P, 1], F32, kind="ExternalInput").ap()
    xs = nc.dram_tensor("xs", [P, DT, N], F32R, kind="ExternalInput").ap()
    ws1 = nc.dram_tensor("ws1", [HST, P, DT * P], F32R, kind="ExternalInput").ap()
    ws11 = nc.dram_tensor("ws11", [HST, P, DT * P], F32R, kind="ExternalInput").ap()
    ws2 = nc.dram_tensor("ws2", [HST, P, D], F32R, kind="ExternalInput").ap()
    bs1 = nc.dram_tensor("bs1", [HST, P, 1], F32, kind="ExternalInput").ap()
    bs11 = nc.dram_tensor("bs11", [HST, P, 1], F32, kind="ExternalInput").ap()
    rout = nc.dram_tensor("rout", [EPC, CT, P, D], F32, kind="ExternalOutput").ap()
    zout = nc.dram_tensor("zout", [N // P, P, D], F32, kind="ExternalOutput").ap()

    mtiles = _mtiles()

    with tile.TileContext(nc) as tc:
        with tc.tile_pool(name="sb", bufs=1) as sb, \
             tc.tile_pool(name="ps", bufs=1, space="PSUM") as ps:

            # --- early loads on the scalar HWDGE queue ---
            xg_t, g_t = [], []
            for j in range(EPC):
                t = sb.tile([P, DT * C], F32R, tag=f"xg{j}", name=f"xg_t{j}")
                nc.scalar.dma_start(t[:], xg[j])
                xg_t.append(t)
                g = sb.tile([P, CT], F32, tag=f"g{j}", name=f"g_t{j}")
                for m in range(CT):
                    nc.scalar.dma_start(g[:, m:m + 1], gt[j, m])
                g_t.append(g)

            ws1_t, ws11_t, ws2_t, bs1_t, bs11_t = [], [], [], [], []
            for hi in range(HST):
                t = sb.tile([P, DT * P], F32R, tag=f"ws1_{hi}", name=f"ws1_t{hi}")
                nc.scalar.dma_start(t[:], ws1[hi])
                ws1_t.append(t)
                t = sb.tile([P, DT * P], F32R, tag=f"ws11_{hi}", name=f"ws11_t{hi}")
                nc.scalar.dma_start(t[:], ws11[hi])
                ws11_t.append(t)
                t = sb.tile([P, D], F32R, tag=f"ws2_{hi}", name=f"ws2_t{hi}")
                nc.scalar.dma_start(t[:], ws2[hi])
                ws2_t.append(t)
                t = sb.tile([P, 1], F32, tag=f"bs1_{hi}", name=f"bs1_t{hi}")
                nc.scalar.dma_start(t[:], bs1[hi])
                bs1_t.append(t)
                t = sb.tile([P, 1], F32, tag=f"bs11_{hi}", name=f"bs11_t{hi}")
                nc.scalar.dma_start(t[:], bs11[hi])
                bs11_t.append(t)

            def routed_expert(j):
                # phase A: hT[hi] = silu(x@W1+b1) * (x@W11+b11), transposed
                h_t = []
                for hi in range(HT):
                    w1_t = sb.tile([P, DT * P], F32R, tag="w1", bufs=2,
                                   name=f"w1_t{j}_{hi}")
                    nc.sync.dma_start(w1_t[:], w1[j, hi])
                    w11_t = sb.tile([P, DT * P], F32R, tag="w11", bufs=2,
                                    name=f"w11_t{j}_{hi}")
                    nc.sync.dma_start(w11_t[:], w11[j, hi])
                    b1_t = sb.tile([P, 1], F32, tag="b1", bufs=3,
                                   name=f"b1_t{j}_{hi}")
                    nc.scalar.dma_start(b1_t[:], b1[j, hi])
                    b11_t = sb.tile([P, 1], F32, tag="b11", bufs=3,
                                    name=f"b11_t{j}_{hi}")
                    nc.scalar.dma_start(b11_t[:], b11[j, hi])

                    ps1 = ps.tile([P, 512], F32, tag="p1", bufs=1,
                                  name=f"ps1_{j}_{hi}")
                    ps2 = ps.tile([P, 512], F32, tag="p2", bufs=1,
                                  name=f"ps2_{j}_{hi}")
                    for ds in range(DT):
                        nc.tensor.matmul(ps1[:, :C],
                                         w1_t[:, ds * P:(ds + 1) * P],
                                         xg_t[j][:, ds * C:(ds + 1) * C],
                                         start=(ds == 0), stop=(ds == DT - 1))
                    for ds in range(DT):
                        nc.tensor.matmul(ps2[:, :C],
                                         w11_t[:, ds * P:(ds + 1) * P],
                                         xg_t[j][:, ds * C:(ds + 1) * C],
                                         start=(ds == 0), stop=(ds == DT - 1))
                    t1 = sb.tile([P, 512], F32, tag="t1", bufs=2,
                                 name=f"t1_{j}_{hi}")
                    nc.scalar.activation(t1[:, :C], ps1[:, :C], AF.Silu,
                                         bias=b1_t[:, 0:1])
                    t2 = sb.tile([P, 512], F32, tag="t2", bufs=2,
                                 name=f"t2_{j}_{hi}")
                    nc.scalar.activation(t2[:, :C], ps2[:, :C], AF.Identity,
                                         bias=b11_t[:, 0:1])
                    ht = sb.tile([P, C], F32R, tag=f"h_{hi}", name=f"h_{j}_{hi}")
                    nc.vector.tensor_mul(ht[:], t1[:, :C], t2[:, :C])
                    h_t.append(ht)

                # phase B: out[m,n] = (h @ W2) * gate ; stream w2 over hi,
                # keep 6 psum accumulators (3 token tiles x 2 D-halves) live
                pos = [[ps.tile([P, 512], F32, tag="po", bufs=6,
                                name=f"po_{j}_{m}_{n2}")
                        for n2 in range(DN)] for m in range(len(mtiles))]
                for hi in range(HT):
                    w2_t = sb.tile([P, D], F32R, tag="w2", bufs=2,
                                   name=f"w2_t{j}_{hi}")
                    nc.sync.dma_start(w2_t[:], w2[j, hi])
                    for m, (off, mw) in enumerate(mtiles):
                        for n2 in range(DN):
                            nc.tensor.matmul(
                                pos[m][n2][:mw, :],
                                h_t[hi][:, off:off + mw],
                                w2_t[:, n2 * 512:(n2 + 1) * 512],
                                start=(hi == 0), stop=(hi == HT - 1))
                for m, (off, mw) in enumerate(mtiles):
                    for n2 in range(DN):
                        o_t = sb.tile([P, 512], F32, tag="ot", bufs=4,
                                      name=f"o_t{j}_{m}_{n2}")
                        nc.vector.tensor_scalar_mul(
                            o_t[:mw, :], pos[m][n2][:mw, :], g_t[j][:mw, m:m + 1])
                        nc.scalar.dma_start(
                            rout[j, m, 0:mw, n2 * 512:(n2 + 1) * 512],
                            o_t[:mw, :])

            def shared_chunk(t):
                xs_t = sb.tile([P, DT * SCH], F32R, tag="xs", bufs=2,
                               name=f"xs_t{t}")
                nc.scalar.dma_start(
                    xs_t[:].rearrange("p (ds s) -> p ds s", ds=DT),
                    xs[:, :, t * SCH:(t + 1) * SCH])
                s_t = []
                for hi in range(HST):
                    ps1 = ps.tile([P, 512], F32, tag="p1", bufs=1,
                                  name=f"sps1_{t}_{hi}")
                    ps2 = ps.tile([P, 512], F32, tag="p2", bufs=1,
                                  name=f"sps2_{t}_{hi}")
                    for ds in range(DT):
                        nc.tensor.matmul(ps1[:],
                                         ws1_t[hi][:, ds * P:(ds + 1) * P],
                                         xs_t[:, ds * SCH:(ds + 1) * SCH],
                                         start=(ds == 0), stop=(ds == DT - 1))
                    for ds in range(DT):
                        nc.tensor.matmul(ps2[:],
                                         ws11_t[hi][:, ds * P:(ds + 1) * P],
                                         xs_t[:, ds * SCH:(ds + 1) * SCH],
                                         start=(ds == 0), stop=(ds == DT - 1))
                    t1 = sb.tile([P, 512], F32, tag="t1", bufs=2,
                                 name=f"st1_{t}_{hi}")
                    nc.scalar.activation(t1[:], ps1[:], AF.Silu,
                                         bias=bs1_t[hi][:, 0:1])
                    t2 = sb.tile([P, 512], F32, tag="t2", bufs=2,
                                 name=f"st2_{t}_{hi}")
                    nc.scalar.activation(t2[:], ps2[:], AF.Identity,
                                         bias=bs11_t[hi][:, 0:1])
                    st = sb.tile([P, 512], F32R, tag=f"s_{hi}", bufs=2,
                                 name=f"s_{t}_{hi}")
                    nc.vector.tensor_mul(st[:], t1[:], t2[:])
                    s_t.append(st)
                for mm in range(SCH // P):
                    for n2 in range(DN):
                        po = ps.tile([P, 512], F32, tag="po", bufs=6,
                                     name=f"spo_{t}_{mm}_{n2}")
                        for hi in range(HST):
                            nc.tensor.matmul(
                                po[:],
                                s_t[hi][:, mm * P:(mm + 1) * P],
                                ws2_t[hi][:, n2 * 512:(n2 + 1) * 512],
                                start=(hi == 0), stop=(hi == HST - 1))
                        z_t = sb.tile([P, 512], F32, tag="zt", bufs=4,
                                      name=f"z_t{t}_{mm}_{n2}")
                        nc.vector.tensor_copy(z_t[:], po[:])
                        nc.scalar.dma_start(
                            zout[t * (SCH // P) + mm, :, n2 * 512:(n2 + 1) * 512],
                            z_t[:])

            routed_expert(0)
            shared_chunk(0)
            shared_chunk(1)
            routed_expert(1)
            shared_chunk(2)
            shared_chunk(3)

    nc.compile()
    return nc


def _route(xf, Wg):
    """Host router: returns (top-k expert ids, gates) per token."""
    logits = xf.astype(np.float64) @ Wg.astype(np.float64)        # [N, E]
    part = np.argpartition(-logits, K - 1, axis=1)[:, :K]          # [N, K]
    pl = np.take_along_axis(logits, part, axis=1)
    order = np.argsort(-pl, axis=1, kind="stable")
    topi = np.take_along_axis(part, order, axis=1)                 # [N, K] sorted
    tl = np.take_along_axis(logits, topi, axis=1)
    m = tl.max(axis=1, keepdims=True)
    e = np.exp(tl - m)
    gates = (e / e.sum(axis=1, keepdims=True)).astype(np.float32)  # [N, K]
    return topi, gates


def kernel(x, Wg, W1, b1, W11, b11, W2, b2, Ws1, bs1, Ws11, bs11, Ws2, bs2,
           _run_opts=None):
    xf = np.ascontiguousarray(x.reshape(N, D), dtype=np.float32)
    topi, gates = _route(xf, Wg)

    # token lists per expert
    flat_e = topi.reshape(-1)                        # [N*K]
    flat_tok = np.repeat(np.arange(N), K)
    flat_g = gates.reshape(-1)
    order = np.argsort(flat_e, kind="stable")
    counts = np.bincount(flat_e, minlength=E)
    starts = np.zeros(E + 1, np.int64)
    np.cumsum(counts, out=starts[1:])
    tok_sorted = flat_tok[order]
    g_sorted = flat_g[order]

    xf_r = _round_tf32(xf)
    xs_arr = xf_r.reshape(N, DT, P).transpose(2, 1, 0).copy()

    in_maps = []
    meta = []          # (expert, idx, g) per (core, j)
    overflow = []      # (expert, idx, g) computed on host
    for c in range(NCORES):
        im = {}
        xg_arr = np.zeros((EPC, P, DT * C), np.float32)
        gt_arr = np.zeros((EPC, CT, P, 1), np.float32)
        w1_arr = np.empty((EPC, HT, P, DT * P), np.float32)
        w11_arr = np.empty((EPC, HT, P, DT * P), np.float32)
        w2_arr = np.empty((EPC, HT, P, D), np.float32)
        b1_arr = np.empty((EPC, HT, P, 1), np.float32)
        b11_arr = np.empty((EPC, HT, P, 1), np.float32)
        core_meta = []
        for j in range(EPC):
            e_id = c * EPC + j
            idx = tok_sorted[starts[e_id]:starts[e_id + 1]]
            g = g_sorted[starts[e_id]:starts[e_id + 1]]
            if len(idx) > C:
                overflow.append((e_id, idx[C:], g[C:]))
                idx, g = idx[:C], g[:C]
            n_e = len(idx)
            core_meta.append((e_id, idx, g))
            # gathered tokens, transposed: [P, DT, C]
            xpad = np.zeros((C, D), np.float32)
            xpad[:n_e] = xf_r[idx]
            xg_arr[j] = xpad.reshape(C, DT, P).transpose(2, 1, 0).reshape(P, DT * C)
            gpad = np.zeros(CT * P, np.float32)
            gpad[:n_e] = g
            gt_arr[j] = gpad.reshape(CT, P, 1)
            w1_arr[j] = _round_tf32(
                W1[e_id].reshape(DT, P, HT, P).transpose(2, 1, 0, 3)
            ).reshape(HT, P, DT * P)
            w11_arr[j] = _round_tf32(
                W11[e_id].reshape(DT, P, HT, P).transpose(2, 1, 0, 3)
            ).reshape(HT, P, DT * P)
            w2_arr[j] = _round_tf32(W2[e_id].reshape(HT, P, D))
            b1_arr[j] = np.asarray(b1[e_id], np.float32).reshape(HT, P, 1)
            b11_arr[j] = np.asarray(b11[e_id], np.float32).reshape(HT, P, 1)
        meta.append(core_meta)
        im["xg"] = xg_arr
        im["gt"] = gt_arr
        im["w1"] = w1_arr
        im["w11"] = w11_arr
        im["w2"] = w2_arr
        im["b1"] = b1_arr
        im["b11"] = b11_arr
        # shared expert slice
        sl = slice(c * HSS, (c + 1) * HSS)
        im["xs"] = xs_arr
        im["ws1"] = _round_tf32(
            np.asarray(Ws1)[:, sl].reshape(DT, P, HST, P).transpose(2, 1, 0, 3)
        ).reshape(HST, P, DT * P)
        im["ws11"] = _round_tf32(
            np.asarray(Ws11)[:, sl].reshape(DT, P, HST, P).transpose(2, 1, 0, 3)
        ).reshape(HST, P, DT * P)
        im["ws2"] = _round_tf32(np.asarray(Ws2)[sl].reshape(HST, P, D))
        im["bs1"] = np.asarray(bs1, np.float32)[sl].reshape(HST, P, 1)
        im["bs11"] = np.asarray(bs11, np.float32)[sl].reshape(HST, P, 1)
        in_maps.append(im)

    if "nc" not in _CACHED:
        _CACHED["nc"] = _build_nc()
    nc = _CACHED["nc"]

    run_opts = _run_opts or {}
    res = bass_utils.run_bass_kernel_spmd(
        nc, in_maps, core_ids=list(range(NCORES)), **run_opts)
    _CACHED["last_results"] = res

    # ---- host-side unshard / combine ----
    y = np.zeros((N, D), np.float32)
    for c in range(NCORES):
        ro = res.results[c]["rout"].reshape(EPC, CT * P, D)
        for j in range(EPC):
            e_id, idx, g = meta[c][j]
            n_e = len(idx)
            np.add.at(y, idx, ro[j, :n_e] + g[:, None] * b2[e_id][None, :])
        if c == 0:
            z = res.results[c]["zout"].reshape(N, D).copy()
        else:
            z += res.results[c]["zout"].reshape(N, D)

    for e_id, idx, g in overflow:
        xo = xf[idx]
        h = _silu(xo @ W1[e_id] + b1[e_id]) * (xo @ W11[e_id] + b11[e_id])
        np.add.at(y, idx, (h @ W2[e_id] + b2[e_id]) * g[:, None])

    out = y + z + np.asarray(bs2, np.float32)[None, :]
    return out.reshape(B, T, D).astype(np.float32)


def _silu(v):
    return v * (1.0 / (1.0 + np.exp(-v)))
